# revision 25
# baseline (speedup 1.0000x reference)
"""TRN2 Bass kernel for nn_FFTMLP_86904368267649.

Reference math: energies[b,o] = sum_f xr[b,f]*w_r[o,f] + xi[b,f]*w_i[o,f]
with w_r = fr+fi, w_i = fr-fi, x: [B, 2, F] fp32, filters: [O, F] fp32.

Structure exploited (three levels):
 1. Filter periodicity (period O=1024 in f): the F=2049 contraction folds
    to T=1024 per channel (f, f+1024, and the f=2048 wrap share columns).
 2. DFT o-reflection: with U = xr'+xi', V = xr'-xi' and
    C[t,o]=0.02cos(2pi o t/1024), S[t,o]=0.02sin(...):
      energies[:, o]      = U@C + V@S         o = 0..511   (E+)
      energies[:, 1024-o] = U@C - V@S                      (E-)
 3. t-reflection: C[1024-t,o]=C[t,o], S[1024-t,o]=-S[t,o], so with
    Ut[t] = U[t]+U[1024-t], Vt[t] = V[t]-V[1024-t] (t=1..511; row 0
    carries U[0], whose C-row is the constant 0.02), the contraction
    halves again to k=512.  The leftover t=512 term U[512]*0.02*(-1)^o
    rides as a k=4 matmul on the raw edge rows (weights replicated).
    The single o=512 output column is computed on the host (one matvec).

Every final k-row Ut/Vt is a +/- combination of 8 raw x rows; the host
ships the raw rows permuted into 8 class slots per quarter (pure gather,
plus zero/wrap fill at t=0).  Each class lands in its own pooled tile
(pool depth 12, sized so the posts never stall the stream yet the
SWDGE in-flight window stays bounded); DVE folds incrementally with the
minimal 8 binary ops per quarter (m = s0+s1+s4+s5, n = s2+s3+s6+s7,
Ut/Vt = m+-n).  The last quarter is restructured: w1/w2 = m +- (s2+s3)
complete two transfers early and only p67 = s6+s7 plus the per-kt-half
Ut = w1+p67, Vt = w2-p67 sit behind the end of the stream, so the final
matmul wave starts ~2us after the last x byte.

The device ships the raw banks P = Ut@C (+edge term) and Q = Vt@S; the
host assembles S1/S2 = P+-Q and unscrambles (reversal) in the gather.
Waves run subtile-major so ACT drains + out DMAs pipeline inside the
wave; a short warmup ramp plus fold-paced keep-alive matmuls hold the
PE HAM clock-gate at 8/8 throughout.

Everything on the wire is bf16 (PSUM accumulates f32): ~21.8 MB/core.
Sharding: data-parallel over batch, 2048 rows per core across 8 cores.
"""

import sys

if "/opt/trn_rl_repo" not in sys.path:
    sys.path.insert(0, "/opt/trn_rl_repo")

import numpy as np
import ml_dtypes

import concourse.bass as bass
import concourse.mybir as mybir
import concourse.tile as tile
from concourse import bacc
from concourse.bass_utils import run_bass_kernel_spmd

BF16NP = ml_dtypes.bfloat16
B, O, F, T = 16384, 1024, 2049, 1024
NCORES = 8
BS = B // NCORES          # 2048 batch rows per core
KT = 4                    # k-tiles over the twice-folded t contraction (512)
OC = 512                  # o-columns per PSUM bank
BQ = 4                    # b-quarters (wave granularity)
QW = BS // BQ             # 512 b-cols per quarter
BSUB = 4                  # 128-row b-subtiles per quarter
NWARM = 20                # initial PE ramp matmuls
F32 = mybir.dt.float32
BF16 = mybir.dt.bfloat16

_CACHE = {}
LAST_RESULTS = None

# DRAM class slots per quarter (emission order; m-side = 0,1,4,5):
#  0: xr[t]        1: xr[t+1024]   2: xr[1024-t]   3: xr[2048-t]
#  4: xi[1024-t]   5: xi[2048-t]   6: xi[t]        7: xi[t+1024]
# m = s0+s1+s4+s5,  n = s2+s3+s6+s7,  Ut = m+n, Vt = m-n.


def _build():
    nc = bacc.Bacc("TRN2", target_bir_lowering=False, debug=False,
                   num_devices=NCORES)

    xt_dram = nc.dram_tensor("xT", [BQ * 8 * 128, KT * QW], BF16,
                             kind="ExternalInput")
    w_dram = nc.dram_tensor("w", [128 * KT, 2 * OC], BF16,
                            kind="ExternalInput")
    # galt[j, o] = 0.02*(-1)^o  (t=512 edge rows -> P bank)
    ga_dram = nc.dram_tensor("ga", [4, OC], BF16, kind="ExternalInput")
    # edge rows: xr[512], xr[1536], xi[512], xi[1536] per b
    e_dram = nc.dram_tensor("edge", [4, BS], BF16, kind="ExternalInput")
    # out rows = b, cols = [P | Q]
    out_dram = nc.dram_tensor("out", [BS, 2 * OC], BF16, kind="ExternalOutput")

    ta = nc.vector.tensor_add
    ts = nc.vector.tensor_sub

    with tile.TileContext(nc) as tc:
        with (
            tc.tile_pool(name="const", bufs=1) as const,
            tc.tile_pool(name="cls", bufs=12) as clsp,
            tc.tile_pool(name="acc", bufs=2) as accp,
            tc.tile_pool(name="outp", bufs=2) as outp,
            tc.tile_pool(name="psum", bufs=8, space="PSUM") as psum,
        ):
            xta = xt_dram.ap().rearrange("(q c p) (k s) -> q c p k s",
                                         q=BQ, c=8, k=KT)
            w_ap = w_dram.ap().rearrange("(p k) o -> p k o", k=KT)
            out_ap = out_dram.ap()

            gat = const.tile([4, OC], BF16)
            nc.sync.dma_start(gat[:], ga_dram.ap())
            et = const.tile([4, BS], BF16)
            nc.sync.dma_start(et[:], e_dram.ap())

            scr = const.tile([128, OC], BF16)
            wt = const.tile([128, KT, 2 * OC], BF16)
            u = const.tile([128, BQ, KT, QW], BF16)
            v = const.tile([128, BQ, KT, QW], BF16)

            warm = psum.tile([128, OC], F32, tag="ps", name="warm")

            def ka(lhs):
                # HAM keep-alive: garbage matmul gated on `lhs`'s producer
                nc.tensor.matmul(warm[:], lhs, scr[:], start=True, stop=True,
                                 skip_group_check=True)

            for bq in range(BQ):
                qs = bq * QW
                # --- x stream: 8 plain class transfers, pooled tiles ------
                # last quarter streams its m-side classes first so m (and
                # then w1/w2) complete before the stream ends; only the
                # n01/p67 tail ops trail the final transfers
                order = [0, 1, 4, 5, 2, 3, 6, 7] if bq == BQ - 1 \
                    else [0, 1, 2, 3, 4, 5, 6, 7]
                c = [None] * 8
                for j in order:
                    t = clsp.tile([128, KT, QW], BF16, tag="cls",
                                  name=f"c{bq}_{j}")
                    nc.gpsimd.dma_start(t[:], xta[bq, j])
                    c[j] = t
                    if bq == 0 and j == 0:
                        # scratch zeros for the warmup ramp; first x post is
                        # already out so the stream head is not delayed
                        nc.gpsimd.memset(scr[:], 0)
                        for i in range(NWARM):
                            nc.tensor.matmul(warm[:], scr[:, 0:128], scr[:],
                                             start=True, stop=True,
                                             skip_group_check=True)
                if bq == 0:
                    # weights ride the same queue behind the first quarter
                    nc.gpsimd.dma_start(wt[:], w_ap)

                # --- fold (minimal binary tree, stream-paced) -------------
                m = accp.tile([128, KT, QW], BF16, tag="m", name=f"m{bq}")
                n = accp.tile([128, KT, QW], BF16, tag="n", name=f"n{bq}")
                if bq < BQ - 1:
                    ta(out=m[:], in0=c[0][:], in1=c[1][:])
                    ta(out=n[:], in0=c[2][:], in1=c[3][:])
                    if bq == 0:
                        for j in range(2, 8):   # stream-paced keep-alives
                            ka(c[j][:, 0, 0:128])
                    ta(out=m[:], in0=m[:], in1=c[4][:])
                    ta(out=m[:], in0=m[:], in1=c[5][:])
                    if bq > 0:
                        ka(m[:, 0, 0:128])      # fires ~60% into the fill
                    ta(out=n[:], in0=n[:], in1=c[6][:])
                    if bq > 0:
                        ka(n[:, 0, 0:128])      # ~80%
                        ka(c[7][:, 0, 0:128])   # ~95% (transfer-gated)
                    ta(out=n[:], in0=n[:], in1=c[7][:])
                    ta(out=u[:, bq], in0=m[:], in1=n[:])
                    ts(out=v[:, bq], in0=m[:], in1=n[:])
                else:
                    # last quarter (m-classes streamed first): the whole fold
                    # is split by b-halves into two independent chains.  The
                    # pre-c7 ops of both halves interleave during the fill;
                    # after the final transfer only p67/Ut/Vt of b-half 0
                    # (3 small ops) gate subtiles 0-1, whose matmuls then
                    # hide b-half 1's tail ops.
                    w1 = accp.tile([128, KT, QW], BF16, tag="w1", name="w1")
                    w2 = accp.tile([128, KT, QW], BF16, tag="w2", name="w2")
                    p67 = accp.tile([128, KT, QW], BF16, tag="p67",
                                    name="p67")
                    H = QW // 2
                    bsl = [slice(0, H), slice(H, QW)]
                    for b_ in bsl:
                        ta(out=m[:, :, b_], in0=c[0][:, :, b_],
                           in1=c[1][:, :, b_])
                    for b_ in bsl:
                        ta(out=m[:, :, b_], in0=m[:, :, b_],
                           in1=c[4][:, :, b_])
                    for b_ in bsl:
                        ta(out=m[:, :, b_], in0=m[:, :, b_],
                           in1=c[5][:, :, b_])
                    ka(m[:, 0, 0:128])          # ~50% into the fill
                    for b_ in bsl:
                        ta(out=n[:, :, b_], in0=c[2][:, :, b_],
                           in1=c[3][:, :, b_])
                    ka(c[3][:, 0, 0:128])       # ~75% (transfer-gated)
                    for b_ in bsl:
                        ta(out=w1[:, :, b_], in0=m[:, :, b_], in1=n[:, :, b_])
                        ts(out=w2[:, :, b_], in0=m[:, :, b_], in1=n[:, :, b_])
                    ka(c[7][:, 0, 0:128])       # ~95%
                    for bi, b_ in enumerate(bsl):
                        ta(out=p67[:, :, b_], in0=c[6][:, :, b_],
                           in1=c[7][:, :, b_])
                        ta(out=u[:, bq, :, b_], in0=w1[:, :, b_],
                           in1=p67[:, :, b_])
                        ts(out=v[:, bq, :, b_], in0=w2[:, :, b_],
                           in1=p67[:, :, b_])
                        if bi == 0:
                            # bridge the post-stream HAM gap so the final
                            # wave starts at full clock
                            ka(u[:, bq, 0, 0:128])
                            ka(v[:, bq, 0, 0:128])

                # --- matmul wave, subtile-major; drains pipeline inside ---
                ps_p = [psum.tile([128, OC], F32, tag="ps",
                                  name=f"psp{bq}_{s}") for s in range(BSUB)]
                ps_q = [psum.tile([128, OC], F32, tag="ps",
                                  name=f"psq{bq}_{s}") for s in range(BSUB)]
                ot = outp.tile([128, BSUB, 2 * OC], BF16, tag="out",
                               name=f"ot{bq}")
                for s in range(BSUB):
                    b0 = s * 128
                    for kt in range(KT):
                        st = (kt == 0)
                        lv = v[:, bq, kt, b0:b0 + 128]
                        lu = u[:, bq, kt, b0:b0 + 128]
                        nc.tensor.matmul(ps_q[s][:], lv, wt[:, kt, OC:],
                                         start=st, stop=(kt == KT - 1),
                                         skip_group_check=True)
                        nc.tensor.matmul(ps_p[s][:], lu, wt[:, kt, :OC],
                                         start=st, stop=False,
                                         skip_group_check=True)
                    eb = et[0:4, qs + b0:qs + b0 + 128]
                    nc.tensor.matmul(ps_p[s][:], eb, gat[:],
                                     start=False, stop=True,
                                     skip_group_check=True)
                    nc.scalar.copy(ot[:, s, 0:OC], ps_p[s][:])
                    if bq == BQ - 1:
                        # DVE is idle by now; drain Q in parallel with ACT
                        nc.vector.tensor_copy(ot[:, s, OC:], ps_q[s][:])
                    else:
                        nc.scalar.copy(ot[:, s, OC:], ps_q[s][:])
                    if bq == BQ - 1:
                        nc.sync.dma_start(out_ap[qs + b0:qs + b0 + 128, :],
                                          ot[:, s])
                if bq < BQ - 1:
                    dst = out_ap[qs:qs + QW, :].rearrange(
                        "(s p) o -> p s o", s=BSUB)
                    nc.sync.dma_start(dst, ot[:])

    nc.compile()
    return nc


# host-side class gather indices (built once)
def _class_indices():
    t = np.arange(512)
    idx = np.zeros((8, 512), np.int64)
    valid = np.ones((8, 512), bool)
    idx[0] = t                      # xr[t]
    idx[1] = t + 1024               # xr[t+1024]
    idx[2][1:] = 1024 - t[1:]       # xr[1024-t]; t=0 -> zero
    valid[2][0] = False
    idx[3][1:] = 2048 - t[1:]       # xr[2048-t]; t=0 -> wrap xr[2048]
    idx[3][0] = 2048
    idx[4][1:] = 1024 - t[1:]       # xi[1024-t]; t=0 -> zero
    valid[4][0] = False
    idx[5][1:] = 2048 - t[1:]       # xi[2048-t]; t=0 -> wrap xi[2048]
    idx[5][0] = 2048
    idx[6] = t                      # xi[t]
    idx[7] = t + 1024               # xi[t+1024]
    ch = np.array([0, 0, 0, 0, 1, 1, 1, 1])
    return idx, valid, ch


_IDX, _VALID, _CH = _class_indices()


def kernel(x, filters_real, filters_imag):
    global LAST_RESULTS
    x = np.asarray(x, dtype=np.float32)
    fr = np.asarray(filters_real, dtype=np.float32)
    fi = np.asarray(filters_imag, dtype=np.float32)

    # C = 0.02cos, S = 0.02sin over [t,o] = [0..511, 0..511] (symmetric),
    # derived from the shipped filters exactly as the reference defines them.
    w_r = fr + fi                             # [O, F]
    w_i = fr - fi
    cfull = 0.5 * (w_r[:, :T] + w_i[:, :T])   # [O, T] = 0.02 cos
    sfull = 0.5 * (w_r[:, :T] - w_i[:, :T])   # [O, T] = 0.02 sin
    wto = np.empty((KT * 128, 2 * OC), np.float32)   # rows t = 0..511
    wto[:, :OC] = cfull[:OC, :OC].T           # C~[t,o]
    wto[:, OC:] = sfull[:OC, :OC].T           # S~[t,o]
    w_np = np.ascontiguousarray(
        wto.reshape(KT, 128, 2 * OC).transpose(1, 0, 2).reshape(
            128 * KT, 2 * OC)).astype(BF16NP)
    ga_np = np.ascontiguousarray(
        np.broadcast_to(cfull[:OC, OC], (4, OC))).astype(BF16NP)

    # o=512 output column on host: one matvec against the full filters
    e512 = x[:, 0] @ w_r[OC] + x[:, 1] @ w_i[OC]          # [B]

    if "nc" not in _CACHE:
        _CACHE["nc"] = _build()
    nc = _CACHE["nc"]

    xbf = x.astype(BF16NP)                    # [B, 2, F]
    from concurrent.futures import ThreadPoolExecutor

    def _shard(c):
        xs = xbf[c * BS:(c + 1) * BS]         # [2048, 2, 2049]
        cls = np.empty((8, 512, BS), BF16NP)
        for j in range(8):
            xcT = xs[:, _CH[j], :].T          # [2049 f, 2048 b] (view)
            cls[j] = xcT[_IDX[j]]
            if not _VALID[j][0]:
                cls[j][0] = 0
        # cls[j, kt*128+p, q*512+b'] -> xt[q, j, p, kt, b']
        xt = np.ascontiguousarray(
            cls.reshape(8, KT, 128, BQ, QW).transpose(3, 0, 2, 1, 4))
        edge = np.ascontiguousarray(
            np.stack([xs[:, 0, 512], xs[:, 0, 1536],
                      xs[:, 1, 512], xs[:, 1, 1536]], axis=0))
        return xt.reshape(BQ * 8 * 128, KT * QW), edge

    with ThreadPoolExecutor(NCORES) as ex:
        shards = list(ex.map(_shard, range(NCORES)))
    in_maps = [{"xT": shards[c][0], "edge": shards[c][1],
                "w": w_np, "ga": ga_np} for c in range(NCORES)]

    import os
    trace = bool(os.environ.get("BASS_TRACE"))
    if trace:
        try:
            import antenv.axon_hooks  # noqa: F401  (shim from test.py)
        except ImportError:
            trace = False
            os.environ["BASS_NEVER_TRACE"] = "1"
    res = run_bass_kernel_spmd(nc, in_maps, list(range(NCORES)), trace=trace)
    LAST_RESULTS = res

    out = np.empty((B, O), np.float32)

    def _gather(c):
        sc = np.asarray(res.results[c]["out"]).astype(np.float32)
        p, q = sc[:, :OC], sc[:, OC:]
        oc = out[c * BS:(c + 1) * BS]
        oc[:, 0:OC] = p + q                   # E+ (col0 = E[0]; Q col0 = 0)
        oc[:, OC] = e512[c * BS:(c + 1) * BS]
        oc[:, OC + 1:] = (p - q)[:, OC - 1:0:-1]   # E- reversed

    with ThreadPoolExecutor(NCORES) as ex:
        list(ex.map(_gather, range(NCORES)))
    return out


# revision 26
# speedup vs baseline: 1.0839x; 1.0839x over previous
"""TRN2 Bass kernel for nn_FFTMLP_86904368267649.

Reference math: energies[b,o] = sum_f xr[b,f]*w_r[o,f] + xi[b,f]*w_i[o,f]
with w_r = fr+fi, w_i = fr-fi, x: [B, 2, F] fp32, filters: [O, F] fp32.

Structure exploited (three levels):
 1. Filter periodicity (period O=1024 in f): the F=2049 contraction folds
    to T=1024 per channel (f, f+1024, and the f=2048 wrap share columns).
 2. DFT o-reflection: with U = xr'+xi', V = xr'-xi' and
    C[t,o]=0.02cos(2pi o t/1024), S[t,o]=0.02sin(...):
      energies[:, o]      = U@C + V@S         o = 0..511   (E+)
      energies[:, 1024-o] = U@C - V@S                      (E-)
 3. t-reflection: C[1024-t,o]=C[t,o], S[1024-t,o]=-S[t,o], so with
    Ut[t] = U[t]+U[1024-t], Vt[t] = V[t]-V[1024-t] (t=1..511; row 0
    carries U[0], whose C-row is the constant 0.02), the contraction
    halves again to k=512.  The leftover t=512 term U[512]*0.02*(-1)^o
    rides as a k=4 matmul on the raw edge rows (weights replicated).
    The single o=512 output column is computed on the host (one matvec).

Every final k-row Ut/Vt is a +/- combination of 8 raw x rows; the host
ships the raw rows permuted into 8 class slots per quarter (pure gather,
plus zero/wrap fill at t=0).  Each class lands in its own pooled tile
(pool depth 12, sized so the posts never stall the stream yet the
SWDGE in-flight window stays bounded); DVE folds incrementally with the
minimal 8 binary ops per quarter (m = s0+s1+s4+s5, n = s2+s3+s6+s7,
Ut/Vt = m+-n).  The last quarter is restructured: w1/w2 = m +- (s2+s3)
complete two transfers early and only p67 = s6+s7 plus the per-kt-half
Ut = w1+p67, Vt = w2-p67 sit behind the end of the stream, so the final
matmul wave starts ~2us after the last x byte.

The device ships the raw banks P = Ut@C (+edge term) and Q = Vt@S; the
host assembles S1/S2 = P+-Q and unscrambles (reversal) in the gather.
Waves run subtile-major so ACT drains + out DMAs pipeline inside the
wave; a short warmup ramp plus fold-paced keep-alive matmuls hold the
PE HAM clock-gate at 8/8 throughout.

Everything on the wire is bf16 (PSUM accumulates f32): ~21.8 MB/core.
Sharding: data-parallel over batch, 2048 rows per core across 8 cores.
"""

import sys

if "/opt/trn_rl_repo" not in sys.path:
    sys.path.insert(0, "/opt/trn_rl_repo")

import numpy as np
import ml_dtypes

import concourse.bass as bass
import concourse.mybir as mybir
import concourse.tile as tile
from concourse import bacc
from concourse.bass_utils import run_bass_kernel_spmd

BF16NP = ml_dtypes.bfloat16
B, O, F, T = 16384, 1024, 2049, 1024
NCORES = 8
BS = B // NCORES          # 2048 batch rows per core
KT = 4                    # k-tiles over the twice-folded t contraction (512)
OC = 512                  # o-columns per PSUM bank
BQ = 4                    # b-quarters (wave granularity)
QW = BS // BQ             # 512 b-cols per quarter
BSUB = 4                  # 128-row b-subtiles per quarter
NWARM = 12                # initial PE ramp matmuls
F32 = mybir.dt.float32
BF16 = mybir.dt.bfloat16

_CACHE = {}
LAST_RESULTS = None

# DRAM class slots per quarter (emission order; m-side = 0,1,4,5):
#  0: xr[t]        1: xr[t+1024]   2: xr[1024-t]   3: xr[2048-t]
#  4: xi[1024-t]   5: xi[2048-t]   6: xi[t]        7: xi[t+1024]
# m = s0+s1+s4+s5,  n = s2+s3+s6+s7,  Ut = m+n, Vt = m-n.


def _build():
    nc = bacc.Bacc("TRN2", target_bir_lowering=False, debug=False,
                   num_devices=NCORES)

    xt_dram = nc.dram_tensor("xT", [BQ * 8 * 128, KT * QW], BF16,
                             kind="ExternalInput")
    w_dram = nc.dram_tensor("w", [128 * KT, 2 * OC], BF16,
                            kind="ExternalInput")
    # galt[j, o] = 0.02*(-1)^o  (t=512 edge rows -> P bank)
    ga_dram = nc.dram_tensor("ga", [4, OC], BF16, kind="ExternalInput")
    # edge rows: xr[512], xr[1536], xi[512], xi[1536] per b
    e_dram = nc.dram_tensor("edge", [4, BS], BF16, kind="ExternalInput")
    # out rows = b, cols = [P | Q]
    out_dram = nc.dram_tensor("out", [BS, 2 * OC], BF16, kind="ExternalOutput")

    ta = nc.vector.tensor_add
    ts = nc.vector.tensor_sub

    with tile.TileContext(nc) as tc:
        with (
            tc.tile_pool(name="const", bufs=1) as const,
            tc.tile_pool(name="cls", bufs=12) as clsp,
            tc.tile_pool(name="acc", bufs=2) as accp,
            tc.tile_pool(name="outp", bufs=2) as outp,
            tc.tile_pool(name="psum", bufs=8, space="PSUM") as psum,
        ):
            xta = xt_dram.ap().rearrange("(q c p) (k s) -> q c p k s",
                                         q=BQ, c=8, k=KT)
            w_ap = w_dram.ap().rearrange("(p k) o -> p k o", k=KT)
            out_ap = out_dram.ap()

            gat = const.tile([4, OC], BF16)
            nc.sync.dma_start(gat[:], ga_dram.ap())
            et = const.tile([4, BS], BF16)
            nc.sync.dma_start(et[:], e_dram.ap())

            scr = const.tile([128, OC], BF16)
            wt = const.tile([128, KT, 2 * OC], BF16)
            u = const.tile([128, BQ, KT, QW], BF16)
            v = const.tile([128, BQ, KT, QW], BF16)

            warm = psum.tile([128, OC], F32, tag="ps", name="warm")

            def ka(lhs):
                # HAM keep-alive: garbage matmul gated on `lhs`'s producer
                nc.tensor.matmul(warm[:], lhs, scr[:], start=True, stop=True,
                                 skip_group_check=True)

            for bq in range(BQ):
                qs = bq * QW
                # --- x stream: 8 plain class transfers, pooled tiles ------
                # last quarter streams its m-side classes first so m (and
                # then w1/w2) complete before the stream ends; only the
                # n01/p67 tail ops trail the final transfers
                order = [0, 1, 4, 5, 2, 3, 6, 7] if bq == BQ - 1 \
                    else [0, 1, 2, 3, 4, 5, 6, 7]
                c = [None] * 8
                for j in order:
                    t = clsp.tile([128, KT, QW], BF16, tag="cls",
                                  name=f"c{bq}_{j}")
                    nc.gpsimd.dma_start(t[:], xta[bq, j])
                    c[j] = t
                    if bq == 0 and j == 0:
                        # scratch zeros for the warmup ramp; first x post is
                        # already out so the stream head is not delayed
                        nc.gpsimd.memset(scr[:], 0)
                        for i in range(NWARM):
                            nc.tensor.matmul(warm[:], scr[:, 0:128], scr[:],
                                             start=True, stop=True,
                                             skip_group_check=True)
                if bq == 0:
                    # weights ride the same queue behind the first quarter
                    nc.gpsimd.dma_start(wt[:], w_ap)

                # --- fold (minimal binary tree, stream-paced) -------------
                m = accp.tile([128, KT, QW], BF16, tag="m", name=f"m{bq}")
                n = accp.tile([128, KT, QW], BF16, tag="n", name=f"n{bq}")
                if bq < BQ - 1:
                    ta(out=m[:], in0=c[0][:], in1=c[1][:])
                    ta(out=n[:], in0=c[2][:], in1=c[3][:])
                    if bq == 0:
                        for j in range(2, 8):   # stream-paced keep-alives
                            ka(c[j][:, 0, 0:128])
                    ta(out=m[:], in0=m[:], in1=c[4][:])
                    ta(out=m[:], in0=m[:], in1=c[5][:])
                    if bq > 0:
                        ka(m[:, 0, 0:128])      # fires ~60% into the fill
                    ta(out=n[:], in0=n[:], in1=c[6][:])
                    if bq > 0:
                        ka(n[:, 0, 0:128])      # ~80%
                        ka(c[7][:, 0, 0:128])   # ~95% (transfer-gated)
                    ta(out=n[:], in0=n[:], in1=c[7][:])
                    ta(out=u[:, bq], in0=m[:], in1=n[:])
                    ts(out=v[:, bq], in0=m[:], in1=n[:])
                else:
                    # last quarter (m-classes streamed first): the whole fold
                    # is split by b-halves into two independent chains.  The
                    # pre-c7 ops of both halves interleave during the fill;
                    # after the final transfer only p67/Ut/Vt of b-half 0
                    # (3 small ops) gate subtiles 0-1, whose matmuls then
                    # hide b-half 1's tail ops.
                    w1 = accp.tile([128, KT, QW], BF16, tag="w1", name="w1")
                    w2 = accp.tile([128, KT, QW], BF16, tag="w2", name="w2")
                    p67 = accp.tile([128, KT, QW], BF16, tag="p67",
                                    name="p67")
                    H = QW // 2
                    bsl = [slice(0, H), slice(H, QW)]
                    for b_ in bsl:
                        ta(out=m[:, :, b_], in0=c[0][:, :, b_],
                           in1=c[1][:, :, b_])
                    for b_ in bsl:
                        ta(out=m[:, :, b_], in0=m[:, :, b_],
                           in1=c[4][:, :, b_])
                    for b_ in bsl:
                        ta(out=m[:, :, b_], in0=m[:, :, b_],
                           in1=c[5][:, :, b_])
                    ka(m[:, 0, 0:128])          # ~50% into the fill
                    for b_ in bsl:
                        ta(out=n[:, :, b_], in0=c[2][:, :, b_],
                           in1=c[3][:, :, b_])
                    ka(c[3][:, 0, 0:128])       # ~75% (transfer-gated)
                    for b_ in bsl:
                        ta(out=w1[:, :, b_], in0=m[:, :, b_], in1=n[:, :, b_])
                        ts(out=w2[:, :, b_], in0=m[:, :, b_], in1=n[:, :, b_])
                    ka(c[7][:, 0, 0:128])       # ~95%
                    for bi, b_ in enumerate(bsl):
                        ta(out=p67[:, :, b_], in0=c[6][:, :, b_],
                           in1=c[7][:, :, b_])
                        ta(out=u[:, bq, :, b_], in0=w1[:, :, b_],
                           in1=p67[:, :, b_])
                        ts(out=v[:, bq, :, b_], in0=w2[:, :, b_],
                           in1=p67[:, :, b_])

                # --- matmul wave, subtile-major; drains pipeline inside ---
                ps_p = [psum.tile([128, OC], F32, tag="ps",
                                  name=f"psp{bq}_{s}") for s in range(BSUB)]
                ps_q = [psum.tile([128, OC], F32, tag="ps",
                                  name=f"psq{bq}_{s}") for s in range(BSUB)]
                ot = outp.tile([128, BSUB, 2 * OC], BF16, tag="out",
                               name=f"ot{bq}")
                for s in range(BSUB):
                    b0 = s * 128
                    for kt in range(KT):
                        st = (kt == 0)
                        lv = v[:, bq, kt, b0:b0 + 128]
                        lu = u[:, bq, kt, b0:b0 + 128]
                        nc.tensor.matmul(ps_q[s][:], lv, wt[:, kt, OC:],
                                         start=st, stop=(kt == KT - 1),
                                         skip_group_check=True)
                        nc.tensor.matmul(ps_p[s][:], lu, wt[:, kt, :OC],
                                         start=st, stop=False,
                                         skip_group_check=True)
                    eb = et[0:4, qs + b0:qs + b0 + 128]
                    nc.tensor.matmul(ps_p[s][:], eb, gat[:],
                                     start=False, stop=True,
                                     skip_group_check=True)
                    nc.scalar.copy(ot[:, s, 0:OC], ps_p[s][:])
                    if bq == BQ - 1:
                        # DVE is idle by now; drain Q in parallel with ACT
                        nc.vector.tensor_copy(ot[:, s, OC:], ps_q[s][:])
                    else:
                        nc.scalar.copy(ot[:, s, OC:], ps_q[s][:])
                    if bq == BQ - 1:
                        nc.sync.dma_start(out_ap[qs + b0:qs + b0 + 128, :],
                                          ot[:, s])
                if bq < BQ - 1:
                    dst = out_ap[qs:qs + QW, :].rearrange(
                        "(s p) o -> p s o", s=BSUB)
                    nc.sync.dma_start(dst, ot[:])

    nc.compile()
    return nc


# host-side class gather indices (built once)
def _class_indices():
    t = np.arange(512)
    idx = np.zeros((8, 512), np.int64)
    valid = np.ones((8, 512), bool)
    idx[0] = t                      # xr[t]
    idx[1] = t + 1024               # xr[t+1024]
    idx[2][1:] = 1024 - t[1:]       # xr[1024-t]; t=0 -> zero
    valid[2][0] = False
    idx[3][1:] = 2048 - t[1:]       # xr[2048-t]; t=0 -> wrap xr[2048]
    idx[3][0] = 2048
    idx[4][1:] = 1024 - t[1:]       # xi[1024-t]; t=0 -> zero
    valid[4][0] = False
    idx[5][1:] = 2048 - t[1:]       # xi[2048-t]; t=0 -> wrap xi[2048]
    idx[5][0] = 2048
    idx[6] = t                      # xi[t]
    idx[7] = t + 1024               # xi[t+1024]
    ch = np.array([0, 0, 0, 0, 1, 1, 1, 1])
    return idx, valid, ch


_IDX, _VALID, _CH = _class_indices()


def kernel(x, filters_real, filters_imag):
    global LAST_RESULTS
    x = np.asarray(x, dtype=np.float32)
    fr = np.asarray(filters_real, dtype=np.float32)
    fi = np.asarray(filters_imag, dtype=np.float32)

    # C = 0.02cos, S = 0.02sin over [t,o] = [0..511, 0..511] (symmetric),
    # derived from the shipped filters exactly as the reference defines them.
    w_r = fr + fi                             # [O, F]
    w_i = fr - fi
    cfull = 0.5 * (w_r[:, :T] + w_i[:, :T])   # [O, T] = 0.02 cos
    sfull = 0.5 * (w_r[:, :T] - w_i[:, :T])   # [O, T] = 0.02 sin
    wto = np.empty((KT * 128, 2 * OC), np.float32)   # rows t = 0..511
    wto[:, :OC] = cfull[:OC, :OC].T           # C~[t,o]
    wto[:, OC:] = sfull[:OC, :OC].T           # S~[t,o]
    w_np = np.ascontiguousarray(
        wto.reshape(KT, 128, 2 * OC).transpose(1, 0, 2).reshape(
            128 * KT, 2 * OC)).astype(BF16NP)
    ga_np = np.ascontiguousarray(
        np.broadcast_to(cfull[:OC, OC], (4, OC))).astype(BF16NP)

    # o=512 output column on host: one matvec against the full filters
    e512 = x[:, 0] @ w_r[OC] + x[:, 1] @ w_i[OC]          # [B]

    if "nc" not in _CACHE:
        _CACHE["nc"] = _build()
    nc = _CACHE["nc"]

    xbf = x.astype(BF16NP)                    # [B, 2, F]
    from concurrent.futures import ThreadPoolExecutor

    def _shard(c):
        xs = xbf[c * BS:(c + 1) * BS]         # [2048, 2, 2049]
        cls = np.empty((8, 512, BS), BF16NP)
        for j in range(8):
            xcT = xs[:, _CH[j], :].T          # [2049 f, 2048 b] (view)
            cls[j] = xcT[_IDX[j]]
            if not _VALID[j][0]:
                cls[j][0] = 0
        # cls[j, kt*128+p, q*512+b'] -> xt[q, j, p, kt, b']
        xt = np.ascontiguousarray(
            cls.reshape(8, KT, 128, BQ, QW).transpose(3, 0, 2, 1, 4))
        edge = np.ascontiguousarray(
            np.stack([xs[:, 0, 512], xs[:, 0, 1536],
                      xs[:, 1, 512], xs[:, 1, 1536]], axis=0))
        return xt.reshape(BQ * 8 * 128, KT * QW), edge

    with ThreadPoolExecutor(NCORES) as ex:
        shards = list(ex.map(_shard, range(NCORES)))
    in_maps = [{"xT": shards[c][0], "edge": shards[c][1],
                "w": w_np, "ga": ga_np} for c in range(NCORES)]

    import os
    trace = bool(os.environ.get("BASS_TRACE"))
    if trace:
        try:
            import antenv.axon_hooks  # noqa: F401  (shim from test.py)
        except ImportError:
            trace = False
            os.environ["BASS_NEVER_TRACE"] = "1"
    res = run_bass_kernel_spmd(nc, in_maps, list(range(NCORES)), trace=trace)
    LAST_RESULTS = res

    out = np.empty((B, O), np.float32)

    def _gather(c):
        sc = np.asarray(res.results[c]["out"]).astype(np.float32)
        p, q = sc[:, :OC], sc[:, OC:]
        oc = out[c * BS:(c + 1) * BS]
        oc[:, 0:OC] = p + q                   # E+ (col0 = E[0]; Q col0 = 0)
        oc[:, OC] = e512[c * BS:(c + 1) * BS]
        oc[:, OC + 1:] = (p - q)[:, OC - 1:0:-1]   # E- reversed

    with ThreadPoolExecutor(NCORES) as ex:
        list(ex.map(_gather, range(NCORES)))
    return out
